# revision 10
# baseline (speedup 1.0000x reference)
"""Multi-head attention (16 heads, DM=1024, DK=DV=64, S=2048, B=2, causal)
on 8 NeuronCores, sharded 4 heads x 1 batch per core (cores 0-3: batch 0
head-groups 0-3; cores 4-7: batch 1). Host sums the 4 partial outputs per
batch and concatenates.

vs the 2-head x 2-batch layout this halves per-core DMA (12MB in, 4MB out)
and doubles the attention rounds per q-chunk, giving the PE more slack to
hide projection work inside the ACT-bound round stream.

Device schedule (single batch, 4 heads = two head-PAIRS a=(h0,h1),
b=(h2,h3); q-chunks of 512 processed in DESCENDING order so the largest
ACT-bound chunks come first while projection backlog exists):

  - upfront: input DMA for the first two chunks, HAM warmup matmuls,
    projections for the first chunk.
  - per chunk c: two sequential round streams (pair a then pair b), one
    s_k tile per round, descending t for ragged causally-trimmed PV
    accumulation.  Rounds: st[128,1024]=[h_lo|h_hi] scores (two K=64
    matmuls, concurrent PE row groups), one ACT exp per round, DVE mask
    multiply on the aligned diagonal block only, PV accumulate into
    ot0/ot1 [65,512] (65th row = rowsums via ones column in v).
  - FILLERS: next chunk's projections (Q/K/V accumulation groups split
    into 2-matmul quanta), previous chunk's WO items, and deferred
    normalization are popped from a FIFO between rounds, keeping the PE
    busy during each round's exp wait.  psM (2 PSUM banks) rotates
    through proj groups / WO yps / rps with at most one group in flight.
  - psO (2 banks) holds one pair's ot0/ot1; the next pair's first PV
    naturally waits on the previous pair's normalization via pool reuse.
  - normalize: rowsum -> 1/r via Ln + Exp(-x) on ACT (same table set as
    exp); broadcast to 64 partitions via K=1 matmul; fused multiply-
    evacuate to oT bf16 on DVE.
  - WO: y[t*128:,:1024] = [oTa|oTb slice].T @ [woa|wob], 4 accumulating
    matmuls per 512-wide half, evacuate bf16, DMA out.
"""

import numpy as np

S, B, DM, DK, DV, H = 2048, 2, 1024, 64, 64, 16
NCORES = 8
HEADS_PER_CORE = 4
SCALE = 1.0 / np.sqrt(DK)  # 1/8

_CACHE = {}


def build_nc(split_waits=True, trim=True):
    # trim=True uses ragged causally-trimmed PV accumulation (descending
    # s_k order, per-element has_written semantics). Real HW supports it;
    # CoreSim's accumulation model does not, so sim runs use trim=False.
    import concourse.bass as bass
    import concourse.tile as tile
    from concourse import mybir

    f32 = mybir.dt.float32
    bf16 = mybir.dt.bfloat16
    Exp = mybir.ActivationFunctionType.Exp
    Ln = mybir.ActivationFunctionType.Ln
    mult = mybir.AluOpType.mult
    nc = bass.Bass()

    xtq = nc.dram_tensor("xtq", [DM, S], bf16, kind="ExternalInput")
    xtk = nc.dram_tensor("xtk", [DM, S], bf16, kind="ExternalInput")
    xtv = nc.dram_tensor("xtv", [DM, S], bf16, kind="ExternalInput")
    # per pair: [DM, 128] stacked on columns -> [128, 2*DM] sbuf layout
    wq = nc.dram_tensor("wq", [2, DM, 128], bf16, kind="ExternalInput")
    wk = nc.dram_tensor("wk", [2, DM, 128], bf16, kind="ExternalInput")
    wv = nc.dram_tensor("wv", [2, DM, 128], bf16, kind="ExternalInput")
    wo = nc.dram_tensor("wo", [2, 128, DM], bf16, kind="ExternalInput")
    masks = nc.dram_tensor("masks", [4, 128, 512], bf16, kind="ExternalInput")
    y = nc.dram_tensor("y", [S, DM], bf16, kind="ExternalOutput")

    NJ = DM // 128  # 8 contraction chunks
    NC_Q = S // 512  # 4 s_q chunks
    NT = S // 128  # 16 s_k tiles
    VW = 130  # per-s_k-tile v storage: [v_h0(64) | 1 | v_h1(64) | 1]

    with tile.TileContext(nc) as tc:
        with (
            tc.tile_pool(name="const", bufs=1) as const,
            tc.tile_pool(name="xt", bufs=3) as xtp,
            tc.tile_pool(name="qkv", bufs=1) as qkvp,
            tc.tile_pool(name="pt", bufs=4) as ptp,
            tc.tile_pool(name="osb", bufs=1) as osbp,
            tc.tile_pool(name="sm", bufs=2) as smp,
            tc.tile_pool(name="ysbp", bufs=5) as ysbp,
            tc.tile_pool(name="psS", bufs=2, space="PSUM") as psS,
            tc.tile_pool(name="psO", bufs=1, space="PSUM") as psO,
            tc.tile_pool(name="psM", bufs=2, space="PSUM") as psM,
        ):
            # ---- constants ----
            wq_sb = const.tile([128, 2 * DM], bf16)
            wk_sb = const.tile([128, 2 * DM], bf16)
            wv_sb = const.tile([128, 2 * DM], bf16)
            wo_sb = const.tile([128, 2 * DM], bf16)
            masks_sb = const.tile([128, 4 * 512], bf16)
            ones_sb = const.tile([1, 512], bf16)
            for w_sb, w_dram in ((wq_sb, wq), (wk_sb, wk), (wv_sb, wv)):
                nc.sync.dma_start(
                    out=w_sb.rearrange("p (x j c) -> p x j c", x=2, c=128),
                    in_=w_dram[:, :, :].rearrange("x (j p) c -> p x j c", p=128))
            nc.sync.dma_start(
                out=wo_sb.rearrange("p (x c) -> p x c", x=2),
                in_=wo[:, :, :].rearrange("x p c -> p x c"))
            nc.sync.dma_start(out=masks_sb.rearrange("p (d q) -> p d q", q=512),
                              in_=masks[:, :, :].rearrange("d p q -> p d q"))
            nc.vector.memset(ones_sb[:], 1.0)

            fillers = []  # FIFO of small emit-closures (PE quanta etc.)

            def _emit_dmas(dc):
                xq = xtp.tile([128, NJ * 512], bf16, tag="xq")
                xk = xtp.tile([128, NJ * 512], bf16, tag="xk")
                xv = xtp.tile([128, NJ * 512], bf16, tag="xv")
                for xt_sb, xt_dram in ((xq, xtq), (xk, xtk), (xv, xtv)):
                    ov = xt_sb.rearrange("p (j s) -> p j s", s=512)
                    iv = xt_dram.rearrange("(j p) s -> p j s", p=128)
                    for g in range(4):  # 4 DMAs/input -> 12 queues busy
                        nc.sync.dma_start(
                            out=ov[:, 2 * g:2 * g + 2, :],
                            in_=iv[:, 2 * g:2 * g + 2,
                                   dc * 512:(dc + 1) * 512])
                return xq, xk, xv

            # persistent per-pair activation tiles (single batch)
            qT = [qkvp.tile([128, S], bf16, tag=f"qT{x}", name=f"qT{x}") for x in range(2)]
            kT = [qkvp.tile([128, S], bf16, tag=f"kT{x}", name=f"kT{x}") for x in range(2)]
            v_sb = [qkvp.tile([128, NT * VW], bf16, tag=f"v{x}", name=f"v{x}")
                    for x in range(2)]
            oT = [osbp.tile([128, S], bf16, tag=f"oT{x}", name=f"oT{x}") for x in range(2)]
            for x in range(2):
                vv = v_sb[x].rearrange("p (t w) -> p t w", w=VW)
                nc.vector.memset(vv[:, :, 64:65], 1.0)
                nc.vector.memset(vv[:, :, 129:130], 1.0)

            def _proj_fillers(xtiles, dc):
                # Per pair x: Q group (8 matmuls as 4 quanta of 2) +
                # evacuate, K likewise, V (4 u-slices of 8 matmuls).
                # Returns per-pair item lists for forced draining.
                xq, xk, xv = xtiles
                need = {0: [], 1: []}
                for x in range(2):
                    for w_sb, dstl, src in ((wq_sb, qT, xq), (wk_sb, kT, xk)):
                        st8 = {}

                        def q2(jj, w_sb=w_sb, x=x, src=src, st8=st8):
                            if jj == 0:
                                st8["ps"] = psM.tile([128, 512], f32, tag="mm", name="ps")
                            ps = st8["ps"]
                            for j in (jj, jj + 1):
                                nc.tensor.matmul(
                                    ps[:],
                                    w_sb[:, x * DM + j * 128:
                                         x * DM + (j + 1) * 128],
                                    src[:, j * 512:(j + 1) * 512],
                                    start=(j == 0), stop=(j == NJ - 1))

                        def ev(dst=dstl, x=x, st8=st8):
                            nc.vector.tensor_copy(
                                dst[x][:, dc * 512:(dc + 1) * 512],
                                st8["ps"][:])
                        item = [lambda jj=jj, f=q2: f(jj)
                                for jj in range(0, NJ, 2)] + [ev]
                        fillers.append(item)
                        need[x].append(item)
                    st8 = {}

                    def v2q(u, x=x, st8=st8, xv=xv):
                        # one u-slice: 8 matmuls of 128 cols (~0.5us)
                        if u == 0:
                            st8["ps"] = psM.tile([128, 512], f32, tag="mm", name="ps")
                        ps = st8["ps"]
                        for j in range(NJ):
                            nc.tensor.matmul(
                                ps[:, u * 128:(u + 1) * 128],
                                xv[:, j * 512 + u * 128:
                                   j * 512 + (u + 1) * 128],
                                wv_sb[:, x * DM + j * 128:
                                      x * DM + (j + 1) * 128],
                                start=(j == 0), stop=(j == NJ - 1))

                    def vev(x=x, st8=st8):
                        nc.vector.tensor_copy(
                            v_sb[x].rearrange("p (t w) -> p t w", w=VW)
                            [:, 4 * dc:4 * dc + 4, 0:130]
                            .rearrange("p u (h w) -> p u h w", h=2)
                            [:, :, :, 0:64],
                            st8["ps"][:].rearrange(
                                "p (u h w) -> p u h w", u=4, h=2))
                    item = [lambda u=u, f=v2q: f(u)
                            for u in range(4)] + [vev]
                    fillers.append(item)
                    need[x].append(item)
                return need

            def _drain_items(items):
                while any(any(it is f for f in fillers) for it in items):
                    _pop_fillers(4)

            def _emit_norm_now(nx, nc_, not0, not1, nrcp):
                # rps broadcast matmul + fused multiply-evacuate into oT.
                # Emitted directly (never via the FIFO): the next pair's
                # first PV waits on the psO banks this releases, so its
                # instructions must precede that PV in program order.
                rps_ps = psM.tile([128, 512], f32, tag="mm")
                for h in (0, 1):
                    nc.tensor.matmul(rps_ps[h * 64:(h + 1) * 64, :],
                                     ones_sb[0:1, 0:64],
                                     nrcp[0:1, h * 512:(h + 1) * 512],
                                     start=True, stop=True)
                for h, otx in ((0, not0), (1, not1)):
                    rps_sb = smp.tile([64, 512], bf16, tag=f"rps{h}")
                    nc.vector.tensor_copy(rps_sb[:],
                                          rps_ps[h * 64:(h + 1) * 64, :])
                    nc.vector.tensor_tensor(
                        out=oT[nx][h * 64:h * 64 + 64,
                                   nc_ * 512:(nc_ + 1) * 512],
                        in0=otx[0:64, :], in1=rps_sb[:], op=mult)

            def _wo_fillers(wc, tail=False):
                # y rows for chunk wc: 4 t-tiles x 2 halves of 512
                for wt in range(4 * wc, 4 * wc + 4):
                    st8 = {}

                    def wo_half(wm, wt=wt, st8=st8, tail=tail):
                        if wm == 0:
                            st8["ysb"] = ysbp.tile([128, 1024], bf16, tag="ysb", name="ysb")
                        yps = psM.tile([128, 512], f32, tag="mm")
                        for x in range(2):
                            nc.tensor.matmul(
                                yps[:],
                                oT[x][:, wt * 128:(wt + 1) * 128],
                                wo_sb[:, x * DM + wm * 512:
                                      x * DM + (wm + 1) * 512],
                                start=(x == 0), stop=(x == 1))
                        if tail and wm == 1:  # spread drain over ACT too
                            nc.scalar.copy(
                                st8["ysb"][:, wm * 512:(wm + 1) * 512],
                                yps[:])
                        else:
                            nc.vector.tensor_copy(
                                st8["ysb"][:, wm * 512:(wm + 1) * 512],
                                yps[:])
                        if wm == 1:
                            nc.sync.dma_start(
                                out=y[wt * 128:(wt + 1) * 128, :],
                                in_=st8["ysb"][:])
                    fillers.append([lambda wm=wm, f=wo_half: f(wm)
                                    for wm in range(2)])

            def _pop_fillers(n):
                done = 0
                while fillers and done < n:
                    item = fillers[0]
                    item.pop(0)()
                    if not item:
                        fillers.pop(0)
                    done += 1

            # HAM warm-up: dependency-free dummy matmuls so the PE clock
            # is at 8/8 when the first projections arrive (covers the
            # initial input-DMA window; PE would be idle regardless).
            warm_ps = psO.tile([64, 512], f32, tag="ot0")
            for _ in range(30):
                nc.tensor.matmul(warm_ps[:], ones_sb[0:1, 0:64], ones_sb[:],
                                 start=True, stop=True)

            xt_cur = _emit_dmas(0)
            xt_next = _emit_dmas(1)
            # first chunk's projections run upfront (PE-only window,
            # overlapped with the warmup/initial-DMA tail)
            proj_need = _proj_fillers(xt_cur, 0)
            while fillers:
                _pop_fillers(8)
            pending_norm = None

            for c in range(NC_Q):
                if c + 1 < NC_Q:
                    next_need = _proj_fillers(xt_next, c + 1)
                    if c + 2 < NC_Q:
                        xt_next = _emit_dmas(c + 2)
                else:
                    next_need = {0: [], 1: []}

                for x in range(2):  # pair a then pair b round streams
                    # previous pair's norm first: it must precede any
                    # WO-filler pop that reads the oT columns it writes,
                    # and it releases the psO banks this pair's PV needs
                    if pending_norm is not None:
                        _emit_norm_now(*pending_norm)
                        pending_norm = None
                    # this pair's projections must be fully emitted
                    # before its rounds reference qT/kT/v_sb
                    _drain_items(proj_need[x])
                    n_t = 4 * c + 4
                    ot0 = psO.tile([65, 512], f32, tag="ot0")
                    ot1 = psO.tile([65, 512], f32, tag="ot1")
                    # s_k tiles DESCENDING: widest PV first (start=True
                    # clears the bank; later narrower tiles accumulate
                    # where written) -> causally-trimmed ragged PV legal.
                    for t in range(n_t - 1, -1, -1):
                        st = psS.tile([128, 1024], f32, tag="st")
                        soff = max(0, (t - 4 * c) * 128) if trim else 0
                        for h in (0, 1):
                            hp = h * 64
                            nc.tensor.matmul(
                                st[:, h * 512 + soff:(h + 1) * 512],
                                kT[x][hp:hp + 64, t * 128:(t + 1) * 128],
                                qT[x][hp:hp + 64,
                                      c * 512 + soff:(c + 1) * 512],
                                start=True, stop=True)
                        pt = ptp.tile([128, 1024], bf16, tag="pt")
                        nc.scalar.activation(pt[:], st[:], Exp,
                                             scale=float(SCALE))
                        off = soff
                        if t >= 4 * c:  # diagonal tile: mask aligned block
                            dd = t - 4 * c
                            mw = 128 if trim else (dd + 1) * 128
                            for h in (0, 1):
                                nc.vector.tensor_tensor(
                                    out=pt[:, h * 512 + off:
                                           h * 512 + off + mw],
                                    in0=pt[:, h * 512 + off:
                                           h * 512 + off + mw],
                                    in1=masks_sb[:, dd * 512 + off:
                                                 dd * 512 + off + mw],
                                    op=mult)
                        for h, otx in ((0, ot0), (1, ot1)):
                            nc.tensor.matmul(
                                otx[0:65, off:512],
                                v_sb[x][:, t * VW + h * 65:
                                        t * VW + h * 65 + 65],
                                pt[:, h * 512 + off:(h + 1) * 512],
                                start=(t == n_t - 1), stop=(t == 0))
                        _pop_fillers(2)

                    # rowsum -> 1/r on ACT (stays in exp table set);
                    # everything else deferred to fillers.
                    lnr = smp.tile([1, 1024], f32, tag="lnr")
                    nc.scalar.activation(lnr[0:1, 0:512], ot0[64:65, :], Ln)
                    nc.scalar.activation(lnr[0:1, 512:1024], ot1[64:65, :],
                                         Ln)
                    rcp = smp.tile([1, 1024], bf16, tag="rcp")
                    nc.scalar.activation(rcp[:], lnr[:], Exp, scale=-1.0)
                    pending_norm = (x, c, ot0, ot1, rcp)
                # WO for this chunk drains during the next chunk's rounds
                _wo_fillers(c, tail=(c == NC_Q - 1))
                proj_need = next_need
            _emit_norm_now(*pending_norm)  # last pair's norm
            while fillers:  # tail drain
                _pop_fillers(8)
    if split_waits:
        _split_waits(nc, mybir)
    return nc


def _split_waits(nc, mybir):
    """This walrus build encodes at most ONE sync wait per instruction.
    Instructions with a single wait keep it inline (free); only multi-
    wait instructions get the extra waits hoisted onto same-engine NoOps
    issued immediately before — semantically identical: the sequencer
    blocks at the NoOp instead."""
    ctr = [0]
    for fn in nc.m.functions:
        for blk in fn.blocks:
            new_insts = []
            for ins in blk.instructions:
                si = getattr(ins, "sync_info", None)
                waits = list(si.on_wait) if si is not None and si.on_wait else []
                if len(waits) > 1:
                    for w in waits[:-1]:
                        ctr[0] += 1
                        nop = mybir.InstNoOp(name=f"WSPLIT-{ctr[0]}", ins=[], outs=[])
                        nop.engine = ins.engine
                        nop.sync_info = mybir.SyncInfo(on_wait=[w], on_update=[])
                        new_insts.append(nop)
                    ins.sync_info = mybir.SyncInfo(
                        on_wait=[waits[-1]], on_update=list(si.on_update or []))
                new_insts.append(ins)
            blk.instructions = new_insts


def _marshal(Q, K, V, WQ, WK, WV, WO):
    Q = np.asarray(Q, dtype=np.float32)
    K = np.asarray(K, dtype=np.float32)
    V = np.asarray(V, dtype=np.float32)
    WQ = np.asarray(WQ, dtype=np.float32)
    WK = np.asarray(WK, dtype=np.float32)
    WV = np.asarray(WV, dtype=np.float32)
    WO = np.asarray(WO, dtype=np.float32)

    import ml_dtypes
    bf = ml_dtypes.bfloat16
    # [B, DM, S] bf16, one batch slice per core
    xt = [np.ascontiguousarray(X.transpose(1, 2, 0)).astype(bf)
          for X in (Q, K, V)]

    masks = np.zeros((4, 128, 512), dtype=bf)
    kk = np.arange(128)[:, None]
    qq = np.arange(512)[None, :]
    for d in range(4):
        masks[d] = (d * 128 + kk <= qq).astype(bf)

    in_maps = []
    for core in range(NCORES):
        b = core // 4
        h0 = (core % 4) * HEADS_PER_CORE
        wql = np.stack([np.concatenate([WQ[h0 + 2 * x], WQ[h0 + 2 * x + 1]],
                                       axis=1) for x in range(2)]).astype(bf)
        wkl = np.stack([np.concatenate([WK[h0 + 2 * x], WK[h0 + 2 * x + 1]],
                                       axis=1) for x in range(2)]).astype(bf)
        wvl = np.stack([np.concatenate([WV[h0 + 2 * x], WV[h0 + 2 * x + 1]],
                                       axis=1) for x in range(2)]).astype(bf)
        wol = np.stack([WO[(h0 + 2 * x) * DV:(h0 + 2 * x + 2) * DV, :]
                        for x in range(2)]).astype(bf)
        in_maps.append({
            "xtq": xt[0][b], "xtk": xt[1][b], "xtv": xt[2][b],
            "wq": np.ascontiguousarray(wql),
            "wk": np.ascontiguousarray(wkl),
            "wv": np.ascontiguousarray(wvl),
            "wo": np.ascontiguousarray(wol),
            "masks": masks,
        })
    return in_maps


LAST_RESULTS = None


def kernel(Q, K, V, WQ, WK, WV, WO):
    global LAST_RESULTS
    from concourse.bass_utils import run_bass_kernel_spmd

    if "nc" not in _CACHE:
        _CACHE["nc"] = build_nc()
    nc = _CACHE["nc"]

    in_maps = _marshal(Q, K, V, WQ, WK, WV, WO)
    res = run_bass_kernel_spmd(nc, in_maps, core_ids=list(range(NCORES)))
    LAST_RESULTS = res
    out = np.zeros((S, B, DM), dtype=np.float32)
    for core, r in enumerate(res.results):
        out[:, core // 4, :] += np.asarray(r["y"]).astype(np.float32)
    return out


# revision 11
# speedup vs baseline: 1.0221x; 1.0221x over previous
"""Multi-head attention (16 heads, DM=1024, DK=DV=64, S=2048, B=2, causal)
on 8 NeuronCores, sharded 4 heads x 1 batch per core (cores 0-3: batch 0
head-groups 0-3; cores 4-7: batch 1). Host sums the 4 partial outputs per
batch and concatenates.

vs the 2-head x 2-batch layout this halves per-core DMA (12MB in, 4MB out)
and doubles the attention rounds per q-chunk, giving the PE more slack to
hide projection work inside the ACT-bound round stream.

Device schedule (single batch, 4 heads = two head-PAIRS a=(h0,h1),
b=(h2,h3); q-chunks of 512 processed in DESCENDING order so the largest
ACT-bound chunks come first while projection backlog exists):

  - upfront: input DMA for the first two chunks, HAM warmup matmuls,
    projections for the first chunk.
  - per chunk c: two sequential round streams (pair a then pair b), one
    s_k tile per round, descending t for ragged causally-trimmed PV
    accumulation.  Rounds: st[128,1024]=[h_lo|h_hi] scores (two K=64
    matmuls, concurrent PE row groups), one ACT exp per round, DVE mask
    multiply on the aligned diagonal block only, PV accumulate into
    ot0/ot1 [65,512] (65th row = rowsums via ones column in v).
  - FILLERS: next chunk's projections (Q/K/V accumulation groups split
    into 2-matmul quanta), previous chunk's WO items, and deferred
    normalization are popped from a FIFO between rounds, keeping the PE
    busy during each round's exp wait.  psM (2 PSUM banks) rotates
    through proj groups / WO yps / rps with at most one group in flight.
  - psO (2 banks) holds one pair's ot0/ot1; the next pair's first PV
    naturally waits on the previous pair's normalization via pool reuse.
  - normalize: rowsum -> 1/r via Ln + Exp(-x) on ACT (same table set as
    exp); broadcast to 64 partitions via K=1 matmul; fused multiply-
    evacuate to oT bf16 on DVE.
  - WO: y[t*128:,:1024] = [oTa|oTb slice].T @ [woa|wob], 4 accumulating
    matmuls per 512-wide half, evacuate bf16, DMA out.
"""

import numpy as np

S, B, DM, DK, DV, H = 2048, 2, 1024, 64, 64, 16
NCORES = 8
HEADS_PER_CORE = 4
SCALE = 1.0 / np.sqrt(DK)  # 1/8

_CACHE = {}


def build_nc(split_waits=True, trim=True):
    # trim=True uses ragged causally-trimmed PV accumulation (descending
    # s_k order, per-element has_written semantics). Real HW supports it;
    # CoreSim's accumulation model does not, so sim runs use trim=False.
    import concourse.bass as bass
    import concourse.tile as tile
    from concourse import mybir

    f32 = mybir.dt.float32
    bf16 = mybir.dt.bfloat16
    Exp = mybir.ActivationFunctionType.Exp
    Ln = mybir.ActivationFunctionType.Ln
    mult = mybir.AluOpType.mult
    nc = bass.Bass()

    xtq = nc.dram_tensor("xtq", [DM, S], bf16, kind="ExternalInput")
    xtk = nc.dram_tensor("xtk", [DM, S], bf16, kind="ExternalInput")
    xtv = nc.dram_tensor("xtv", [DM, S], bf16, kind="ExternalInput")
    # per pair: [DM, 128] stacked on columns -> [128, 2*DM] sbuf layout
    wq = nc.dram_tensor("wq", [2, DM, 128], bf16, kind="ExternalInput")
    wk = nc.dram_tensor("wk", [2, DM, 128], bf16, kind="ExternalInput")
    wv = nc.dram_tensor("wv", [2, DM, 128], bf16, kind="ExternalInput")
    wo = nc.dram_tensor("wo", [2, 128, DM], bf16, kind="ExternalInput")
    masks = nc.dram_tensor("masks", [4, 128, 512], bf16, kind="ExternalInput")
    y = nc.dram_tensor("y", [S, DM], bf16, kind="ExternalOutput")

    NJ = DM // 128  # 8 contraction chunks
    NC_Q = S // 512  # 4 s_q chunks
    NT = S // 128  # 16 s_k tiles
    VW = 130  # per-s_k-tile v storage: [v_h0(64) | 1 | v_h1(64) | 1]

    with tile.TileContext(nc) as tc:
        with (
            tc.tile_pool(name="const", bufs=1) as const,
            tc.tile_pool(name="xt", bufs=3) as xtp,
            tc.tile_pool(name="qkv", bufs=1) as qkvp,
            tc.tile_pool(name="pt", bufs=4) as ptp,
            tc.tile_pool(name="osb", bufs=1) as osbp,
            tc.tile_pool(name="sm", bufs=2) as smp,
            tc.tile_pool(name="ysbp", bufs=5) as ysbp,
            tc.tile_pool(name="psS", bufs=2, space="PSUM") as psS,
            tc.tile_pool(name="psO", bufs=1, space="PSUM") as psO,
            tc.tile_pool(name="psM", bufs=2, space="PSUM") as psM,
        ):
            # ---- constants ----
            wq_sb = const.tile([128, 2 * DM], bf16)
            wk_sb = const.tile([128, 2 * DM], bf16)
            wv_sb = const.tile([128, 2 * DM], bf16)
            wo_sb = const.tile([128, 2 * DM], bf16)
            masks_sb = const.tile([128, 4 * 512], bf16)
            ones_sb = const.tile([1, 512], bf16)
            for w_sb, w_dram in ((wq_sb, wq), (wk_sb, wk), (wv_sb, wv)):
                nc.sync.dma_start(
                    out=w_sb.rearrange("p (x j c) -> p x j c", x=2, c=128),
                    in_=w_dram[:, :, :].rearrange("x (j p) c -> p x j c", p=128))
            nc.sync.dma_start(
                out=wo_sb.rearrange("p (x c) -> p x c", x=2),
                in_=wo[:, :, :].rearrange("x p c -> p x c"))
            nc.sync.dma_start(out=masks_sb.rearrange("p (d q) -> p d q", q=512),
                              in_=masks[:, :, :].rearrange("d p q -> p d q"))
            nc.vector.memset(ones_sb[:], 1.0)

            fillers = []  # FIFO of small emit-closures (PE quanta etc.)

            def _emit_dmas(dc):
                xq = xtp.tile([128, NJ * 512], bf16, tag="xq")
                xk = xtp.tile([128, NJ * 512], bf16, tag="xk")
                xv = xtp.tile([128, NJ * 512], bf16, tag="xv")
                for xt_sb, xt_dram in ((xq, xtq), (xk, xtk), (xv, xtv)):
                    ov = xt_sb.rearrange("p (j s) -> p j s", s=512)
                    iv = xt_dram.rearrange("(j p) s -> p j s", p=128)
                    for g in range(4):  # 4 DMAs/input -> 12 queues busy
                        nc.sync.dma_start(
                            out=ov[:, 2 * g:2 * g + 2, :],
                            in_=iv[:, 2 * g:2 * g + 2,
                                   dc * 512:(dc + 1) * 512])
                return xq, xk, xv

            # persistent per-pair activation tiles (single batch)
            qT = [qkvp.tile([128, S], bf16, tag=f"qT{x}", name=f"qT{x}") for x in range(2)]
            kT = [qkvp.tile([128, S], bf16, tag=f"kT{x}", name=f"kT{x}") for x in range(2)]
            v_sb = [qkvp.tile([128, NT * VW], bf16, tag=f"v{x}", name=f"v{x}")
                    for x in range(2)]
            oT = [osbp.tile([128, S], bf16, tag=f"oT{x}", name=f"oT{x}") for x in range(2)]
            for x in range(2):
                vv = v_sb[x].rearrange("p (t w) -> p t w", w=VW)
                nc.vector.memset(vv[:, :, 64:65], 1.0)
                nc.vector.memset(vv[:, :, 129:130], 1.0)

            def _proj_fillers(xtiles, dc):
                # Per pair x: Q group (8 matmuls as 4 quanta of 2) +
                # evacuate, K likewise, V (4 u-slices of 8 matmuls).
                # Returns per-pair item lists for forced draining.
                xq, xk, xv = xtiles
                need = {0: [], 1: []}
                for x in range(2):
                    for w_sb, dstl, src in ((wq_sb, qT, xq), (wk_sb, kT, xk)):
                        st8 = {}

                        def q2(jj, w_sb=w_sb, x=x, src=src, st8=st8):
                            if jj == 0:
                                st8["ps"] = psM.tile([128, 512], f32, tag="mm", name="ps")
                            ps = st8["ps"]
                            for j in (jj, jj + 1):
                                nc.tensor.matmul(
                                    ps[:],
                                    w_sb[:, x * DM + j * 128:
                                         x * DM + (j + 1) * 128],
                                    src[:, j * 512:(j + 1) * 512],
                                    start=(j == 0), stop=(j == NJ - 1))

                        def ev(dst=dstl, x=x, st8=st8):
                            nc.vector.tensor_copy(
                                dst[x][:, dc * 512:(dc + 1) * 512],
                                st8["ps"][:])
                        item = [lambda jj=jj, f=q2: f(jj)
                                for jj in range(0, NJ, 2)] + [ev]
                        fillers.append(item)
                        need[x].append(item)
                    st8 = {}

                    def v2q(u, x=x, st8=st8, xv=xv):
                        # one u-slice: 8 matmuls of 128 cols (~0.5us)
                        if u == 0:
                            st8["ps"] = psM.tile([128, 512], f32, tag="mm", name="ps")
                        ps = st8["ps"]
                        for j in range(NJ):
                            nc.tensor.matmul(
                                ps[:, u * 128:(u + 1) * 128],
                                xv[:, j * 512 + u * 128:
                                   j * 512 + (u + 1) * 128],
                                wv_sb[:, x * DM + j * 128:
                                      x * DM + (j + 1) * 128],
                                start=(j == 0), stop=(j == NJ - 1))

                    def vev(x=x, st8=st8):
                        nc.vector.tensor_copy(
                            v_sb[x].rearrange("p (t w) -> p t w", w=VW)
                            [:, 4 * dc:4 * dc + 4, 0:130]
                            .rearrange("p u (h w) -> p u h w", h=2)
                            [:, :, :, 0:64],
                            st8["ps"][:].rearrange(
                                "p (u h w) -> p u h w", u=4, h=2))
                    item = [lambda u=u, f=v2q: f(u)
                            for u in range(4)] + [vev]
                    fillers.append(item)
                    need[x].append(item)
                return need

            def _drain_items(items):
                while any(any(it is f for f in fillers) for it in items):
                    _pop_fillers(1)

            def _emit_norm_now(nx, nc_, not0, not1, nrcp):
                # rps broadcast matmul + fused multiply-evacuate into oT.
                # Emitted directly (never via the FIFO): the next pair's
                # first PV waits on the psO banks this releases, so its
                # instructions must precede that PV in program order.
                rps_ps = psM.tile([128, 512], f32, tag="mm")
                for h in (0, 1):
                    nc.tensor.matmul(rps_ps[h * 64:(h + 1) * 64, :],
                                     ones_sb[0:1, 0:64],
                                     nrcp[0:1, h * 512:(h + 1) * 512],
                                     start=True, stop=True)
                for h, otx in ((0, not0), (1, not1)):
                    rps_sb = smp.tile([64, 512], bf16, tag=f"rps{h}")
                    nc.vector.tensor_copy(rps_sb[:],
                                          rps_ps[h * 64:(h + 1) * 64, :])
                    nc.vector.tensor_tensor(
                        out=oT[nx][h * 64:h * 64 + 64,
                                   nc_ * 512:(nc_ + 1) * 512],
                        in0=otx[0:64, :], in1=rps_sb[:], op=mult)

            def _wo_fillers(wc, tail=False):
                # y rows for chunk wc: 4 t-tiles x 2 halves of 512
                for wt in range(4 * wc, 4 * wc + 4):
                    st8 = {}

                    def wo_half(wm, wt=wt, st8=st8, tail=tail):
                        if wm == 0:
                            st8["ysb"] = ysbp.tile([128, 1024], bf16, tag="ysb", name="ysb")
                        yps = psM.tile([128, 512], f32, tag="mm")
                        for x in range(2):
                            nc.tensor.matmul(
                                yps[:],
                                oT[x][:, wt * 128:(wt + 1) * 128],
                                wo_sb[:, x * DM + wm * 512:
                                      x * DM + (wm + 1) * 512],
                                start=(x == 0), stop=(x == 1))
                        if tail and wm == 1:  # spread drain over ACT too
                            nc.scalar.copy(
                                st8["ysb"][:, wm * 512:(wm + 1) * 512],
                                yps[:])
                        else:
                            nc.vector.tensor_copy(
                                st8["ysb"][:, wm * 512:(wm + 1) * 512],
                                yps[:])
                        if wm == 1:
                            nc.sync.dma_start(
                                out=y[wt * 128:(wt + 1) * 128, :],
                                in_=st8["ysb"][:])
                    fillers.append([lambda wm=wm, f=wo_half: f(wm)
                                    for wm in range(2)])

            def _pop_fillers(n):
                done = 0
                while fillers and done < n:
                    item = fillers[0]
                    item.pop(0)()
                    if not item:
                        fillers.pop(0)
                    done += 1

            # HAM warm-up: dependency-free dummy matmuls so the PE clock
            # is at 8/8 when the first projections arrive (covers the
            # initial input-DMA window; PE would be idle regardless).
            warm_ps = psO.tile([64, 512], f32, tag="ot0")
            for _ in range(20):
                nc.tensor.matmul(warm_ps[:], ones_sb[0:1, 0:64], ones_sb[:],
                                 start=True, stop=True)

            xt_cur = _emit_dmas(0)
            # first chunk's projections run upfront (PE-only window,
            # overlapped with the warmup/initial-DMA tail); chunk 1's
            # input DMA is issued after so it does not steal queue
            # bandwidth from chunk 0's
            proj_need = _proj_fillers(xt_cur, 0)
            while fillers:
                _pop_fillers(8)
            xt_next = _emit_dmas(1)
            pending_norm = None

            for c in range(NC_Q):
                if c + 1 < NC_Q:
                    next_need = _proj_fillers(xt_next, c + 1)
                    if c + 2 < NC_Q:
                        xt_next = _emit_dmas(c + 2)
                else:
                    next_need = {0: [], 1: []}

                for x in range(2):  # pair a then pair b round streams
                    # previous pair's norm first: it must precede any
                    # WO-filler pop that reads the oT columns it writes,
                    # and it releases the psO banks this pair's PV needs
                    if pending_norm is not None:
                        _emit_norm_now(*pending_norm)
                        pending_norm = None
                    # this pair's projections must be fully emitted
                    # before its rounds reference qT/kT/v_sb
                    _drain_items(proj_need[x])
                    n_t = 4 * c + 4
                    ot0 = psO.tile([65, 512], f32, tag="ot0")
                    ot1 = psO.tile([65, 512], f32, tag="ot1")
                    # s_k tiles DESCENDING: widest PV first (start=True
                    # clears the bank; later narrower tiles accumulate
                    # where written) -> causally-trimmed ragged PV legal.
                    for t in range(n_t - 1, -1, -1):
                        st = psS.tile([128, 1024], f32, tag="st")
                        soff = max(0, (t - 4 * c) * 128) if trim else 0
                        for h in (0, 1):
                            hp = h * 64
                            nc.tensor.matmul(
                                st[:, h * 512 + soff:(h + 1) * 512],
                                kT[x][hp:hp + 64, t * 128:(t + 1) * 128],
                                qT[x][hp:hp + 64,
                                      c * 512 + soff:(c + 1) * 512],
                                start=True, stop=True)
                        pt = ptp.tile([128, 1024], bf16, tag="pt")
                        nc.scalar.activation(pt[:], st[:], Exp,
                                             scale=float(SCALE))
                        off = soff
                        if t >= 4 * c:  # diagonal tile: mask aligned block
                            dd = t - 4 * c
                            mw = 128 if trim else (dd + 1) * 128
                            for h in (0, 1):
                                nc.vector.tensor_tensor(
                                    out=pt[:, h * 512 + off:
                                           h * 512 + off + mw],
                                    in0=pt[:, h * 512 + off:
                                           h * 512 + off + mw],
                                    in1=masks_sb[:, dd * 512 + off:
                                                 dd * 512 + off + mw],
                                    op=mult)
                        for h, otx in ((0, ot0), (1, ot1)):
                            nc.tensor.matmul(
                                otx[0:65, off:512],
                                v_sb[x][:, t * VW + h * 65:
                                        t * VW + h * 65 + 65],
                                pt[:, h * 512 + off:(h + 1) * 512],
                                start=(t == n_t - 1), stop=(t == 0))
                        _pop_fillers(3)

                    # rowsum -> 1/r on ACT (stays in exp table set);
                    # everything else deferred to fillers.
                    lnr = smp.tile([1, 1024], f32, tag="lnr")
                    nc.scalar.activation(lnr[0:1, 0:512], ot0[64:65, :], Ln)
                    nc.scalar.activation(lnr[0:1, 512:1024], ot1[64:65, :],
                                         Ln)
                    rcp = smp.tile([1, 1024], bf16, tag="rcp")
                    nc.scalar.activation(rcp[:], lnr[:], Exp, scale=-1.0)
                    pending_norm = (x, c, ot0, ot1, rcp)
                # WO for this chunk drains during the next chunk's rounds
                _wo_fillers(c, tail=(c == NC_Q - 1))
                proj_need = next_need
            _emit_norm_now(*pending_norm)  # last pair's norm
            while fillers:  # tail drain
                _pop_fillers(8)
    if split_waits:
        _split_waits(nc, mybir)
    return nc


def _split_waits(nc, mybir):
    """This walrus build encodes at most ONE sync wait per instruction.
    Instructions with a single wait keep it inline (free); only multi-
    wait instructions get the extra waits hoisted onto same-engine NoOps
    issued immediately before — semantically identical: the sequencer
    blocks at the NoOp instead."""
    ctr = [0]
    for fn in nc.m.functions:
        for blk in fn.blocks:
            new_insts = []
            for ins in blk.instructions:
                si = getattr(ins, "sync_info", None)
                waits = list(si.on_wait) if si is not None and si.on_wait else []
                if len(waits) > 1:
                    for w in waits[:-1]:
                        ctr[0] += 1
                        nop = mybir.InstNoOp(name=f"WSPLIT-{ctr[0]}", ins=[], outs=[])
                        nop.engine = ins.engine
                        nop.sync_info = mybir.SyncInfo(on_wait=[w], on_update=[])
                        new_insts.append(nop)
                    ins.sync_info = mybir.SyncInfo(
                        on_wait=[waits[-1]], on_update=list(si.on_update or []))
                new_insts.append(ins)
            blk.instructions = new_insts


def _marshal(Q, K, V, WQ, WK, WV, WO):
    Q = np.asarray(Q, dtype=np.float32)
    K = np.asarray(K, dtype=np.float32)
    V = np.asarray(V, dtype=np.float32)
    WQ = np.asarray(WQ, dtype=np.float32)
    WK = np.asarray(WK, dtype=np.float32)
    WV = np.asarray(WV, dtype=np.float32)
    WO = np.asarray(WO, dtype=np.float32)

    import ml_dtypes
    bf = ml_dtypes.bfloat16
    # [B, DM, S] bf16, one batch slice per core
    xt = [np.ascontiguousarray(X.transpose(1, 2, 0)).astype(bf)
          for X in (Q, K, V)]

    masks = np.zeros((4, 128, 512), dtype=bf)
    kk = np.arange(128)[:, None]
    qq = np.arange(512)[None, :]
    for d in range(4):
        masks[d] = (d * 128 + kk <= qq).astype(bf)

    in_maps = []
    for core in range(NCORES):
        b = core // 4
        h0 = (core % 4) * HEADS_PER_CORE
        wql = np.stack([np.concatenate([WQ[h0 + 2 * x], WQ[h0 + 2 * x + 1]],
                                       axis=1) for x in range(2)]).astype(bf)
        wkl = np.stack([np.concatenate([WK[h0 + 2 * x], WK[h0 + 2 * x + 1]],
                                       axis=1) for x in range(2)]).astype(bf)
        wvl = np.stack([np.concatenate([WV[h0 + 2 * x], WV[h0 + 2 * x + 1]],
                                       axis=1) for x in range(2)]).astype(bf)
        wol = np.stack([WO[(h0 + 2 * x) * DV:(h0 + 2 * x + 2) * DV, :]
                        for x in range(2)]).astype(bf)
        in_maps.append({
            "xtq": xt[0][b], "xtk": xt[1][b], "xtv": xt[2][b],
            "wq": np.ascontiguousarray(wql),
            "wk": np.ascontiguousarray(wkl),
            "wv": np.ascontiguousarray(wvl),
            "wo": np.ascontiguousarray(wol),
            "masks": masks,
        })
    return in_maps


LAST_RESULTS = None


def kernel(Q, K, V, WQ, WK, WV, WO):
    global LAST_RESULTS
    from concourse.bass_utils import run_bass_kernel_spmd

    if "nc" not in _CACHE:
        _CACHE["nc"] = build_nc()
    nc = _CACHE["nc"]

    in_maps = _marshal(Q, K, V, WQ, WK, WV, WO)
    res = run_bass_kernel_spmd(nc, in_maps, core_ids=list(range(NCORES)))
    LAST_RESULTS = res
    out = np.zeros((S, B, DM), dtype=np.float32)
    for core, r in enumerate(res.results):
        out[:, core // 4, :] += np.asarray(r["y"]).astype(np.float32)
    return out


# revision 13
# speedup vs baseline: 1.0944x; 1.0708x over previous
"""Multi-head attention (16 heads, DM=1024, DK=DV=64, S=2048, B=2, causal)
on 8 NeuronCores, sharded 4 heads x 1 batch per core (cores 0-3: batch 0
head-groups 0-3; cores 4-7: batch 1). Host sums the 4 partial outputs per
batch and concatenates.

vs the 2-head x 2-batch layout this halves per-core DMA (12MB in, 4MB out)
and doubles the attention rounds per q-chunk, giving the PE more slack to
hide projection work inside the ACT-bound round stream.

Device schedule (single batch, 4 heads = two head-PAIRS a=(h0,h1),
b=(h2,h3); q-chunks of 512 processed in DESCENDING order so the largest
ACT-bound chunks come first while projection backlog exists):

  - upfront: input DMA for the first two chunks, HAM warmup matmuls,
    projections for the first chunk.
  - per chunk c: two sequential round streams (pair a then pair b), one
    s_k tile per round, descending t for ragged causally-trimmed PV
    accumulation.  Rounds: st[128,1024]=[h_lo|h_hi] scores (two K=64
    matmuls, concurrent PE row groups), one ACT exp per round, DVE mask
    multiply on the aligned diagonal block only, PV accumulate into
    ot0/ot1 [65,512] (65th row = rowsums via ones column in v).
  - FILLERS: next chunk's projections (Q/K/V accumulation groups split
    into 2-matmul quanta), previous chunk's WO items, and deferred
    normalization are popped from a FIFO between rounds, keeping the PE
    busy during each round's exp wait.  psM (2 PSUM banks) rotates
    through proj groups / WO yps / rps with at most one group in flight.
  - psO (2 banks) holds one pair's ot0/ot1; the next pair's first PV
    naturally waits on the previous pair's normalization via pool reuse.
  - normalize: rowsum -> 1/r via Ln + Exp(-x) on ACT (same table set as
    exp); broadcast to 64 partitions via K=1 matmul; fused multiply-
    evacuate to oT bf16 on DVE.
  - WO: y[t*128:,:1024] = [oTa|oTb slice].T @ [woa|wob], 4 accumulating
    matmuls per 512-wide half, evacuate bf16, DMA out.
"""

import numpy as np

S, B, DM, DK, DV, H = 2048, 2, 1024, 64, 64, 16
NCORES = 8
HEADS_PER_CORE = 4
SCALE = 1.0 / np.sqrt(DK)  # 1/8

_CACHE = {}


def build_nc(split_waits=True, trim=True):
    # trim=True uses ragged causally-trimmed PV accumulation (descending
    # s_k order, per-element has_written semantics). Real HW supports it;
    # CoreSim's accumulation model does not, so sim runs use trim=False.
    import concourse.bass as bass
    import concourse.tile as tile
    from concourse import mybir

    f32 = mybir.dt.float32
    bf16 = mybir.dt.bfloat16
    Exp = mybir.ActivationFunctionType.Exp
    Ln = mybir.ActivationFunctionType.Ln
    mult = mybir.AluOpType.mult
    nc = bass.Bass()

    xtq = nc.dram_tensor("xtq", [DM, S], bf16, kind="ExternalInput")
    xtk = nc.dram_tensor("xtk", [DM, S], bf16, kind="ExternalInput")
    xtv = nc.dram_tensor("xtv", [DM, S], bf16, kind="ExternalInput")
    # per pair: [DM, 128] stacked on columns -> [128, 2*DM] sbuf layout
    wq = nc.dram_tensor("wq", [2, DM, 128], bf16, kind="ExternalInput")
    wk = nc.dram_tensor("wk", [2, DM, 128], bf16, kind="ExternalInput")
    wv = nc.dram_tensor("wv", [2, DM, 128], bf16, kind="ExternalInput")
    wo = nc.dram_tensor("wo", [2, 128, DM], bf16, kind="ExternalInput")
    masks = nc.dram_tensor("masks", [4, 128, 512], bf16, kind="ExternalInput")
    y = nc.dram_tensor("y", [S, DM], bf16, kind="ExternalOutput")

    NJ = DM // 128  # 8 contraction chunks
    NC_Q = S // 512  # 4 s_q chunks
    NT = S // 128  # 16 s_k tiles
    VW = 130  # per-s_k-tile v storage: [v_h0(64) | 1 | v_h1(64) | 1]

    with tile.TileContext(nc) as tc:
        with (
            tc.tile_pool(name="const", bufs=1) as const,
            tc.tile_pool(name="xt", bufs=3) as xtp,
            tc.tile_pool(name="qkv", bufs=1) as qkvp,
            tc.tile_pool(name="pt", bufs=4) as ptp,
            tc.tile_pool(name="osb", bufs=1) as osbp,
            tc.tile_pool(name="sm", bufs=2) as smp,
            tc.tile_pool(name="ysbp", bufs=5) as ysbp,
            tc.tile_pool(name="psS", bufs=2, space="PSUM") as psS,
            tc.tile_pool(name="psO", bufs=1, space="PSUM") as psO,
            tc.tile_pool(name="psM", bufs=2, space="PSUM") as psM,
        ):
            # ---- constants ----
            wq_sb = const.tile([128, 2 * DM], bf16)
            wk_sb = const.tile([128, 2 * DM], bf16)
            wv_sb = const.tile([128, 2 * DM], bf16)
            wo_sb = const.tile([128, 2 * DM], bf16)
            masks_sb = const.tile([128, 4 * 512], bf16)
            ones_sb = const.tile([1, 512], bf16)
            for w_sb, w_dram in ((wq_sb, wq), (wk_sb, wk), (wv_sb, wv)):
                nc.sync.dma_start(
                    out=w_sb.rearrange("p (x j c) -> p x j c", x=2, c=128),
                    in_=w_dram[:, :, :].rearrange("x (j p) c -> p x j c", p=128))
            nc.vector.memset(ones_sb[:], 1.0)

            def _late_const_dmas():
                # wo/masks are not needed until the first rounds/WO --
                # issued after chunk 0's input DMA so they don't steal
                # queue bandwidth from the critical startup path
                nc.sync.dma_start(
                    out=wo_sb.rearrange("p (x c) -> p x c", x=2),
                    in_=wo[:, :, :].rearrange("x p c -> p x c"))
                nc.sync.dma_start(
                    out=masks_sb.rearrange("p (d q) -> p d q", q=512),
                    in_=masks[:, :, :].rearrange("d p q -> p d q"))

            fillers = []  # FIFO of small emit-closures (PE quanta etc.)

            def _emit_dmas(dc):
                xq = xtp.tile([128, NJ * 512], bf16, tag="xq")
                xk = xtp.tile([128, NJ * 512], bf16, tag="xk")
                xv = xtp.tile([128, NJ * 512], bf16, tag="xv")
                for xt_sb, xt_dram in ((xq, xtq), (xk, xtk), (xv, xtv)):
                    ov = xt_sb.rearrange("p (j s) -> p j s", s=512)
                    iv = xt_dram.rearrange("(j p) s -> p j s", p=128)
                    for g in range(4):  # 4 DMAs/input -> 12 queues busy
                        nc.sync.dma_start(
                            out=ov[:, 2 * g:2 * g + 2, :],
                            in_=iv[:, 2 * g:2 * g + 2,
                                   dc * 512:(dc + 1) * 512])
                return xq, xk, xv

            # persistent per-pair activation tiles (single batch)
            qT = [qkvp.tile([128, S], bf16, tag=f"qT{x}", name=f"qT{x}") for x in range(2)]
            kT = [qkvp.tile([128, S], bf16, tag=f"kT{x}", name=f"kT{x}") for x in range(2)]
            v_sb = [qkvp.tile([128, NT * VW], bf16, tag=f"v{x}", name=f"v{x}")
                    for x in range(2)]
            oT = [osbp.tile([128, S], bf16, tag=f"oT{x}", name=f"oT{x}") for x in range(2)]
            for x in range(2):
                vv = v_sb[x].rearrange("p (t w) -> p t w", w=VW)
                nc.vector.memset(vv[:, :, 64:65], 1.0)
                nc.vector.memset(vv[:, :, 129:130], 1.0)

            def _proj_fillers(xtiles, dc):
                # Per pair x: Q group (8 matmuls as 4 quanta of 2) +
                # evacuate, K likewise, V (4 u-slices of 8 matmuls).
                # Returns per-pair item lists for forced draining.
                xq, xk, xv = xtiles
                need = {0: [], 1: []}
                for x in range(2):
                    for w_sb, dstl, src in ((wq_sb, qT, xq), (wk_sb, kT, xk)):
                        st8 = {}

                        def q2(jj, w_sb=w_sb, x=x, src=src, st8=st8):
                            if jj == 0:
                                st8["ps"] = psM.tile([128, 512], f32, tag="mm", name="ps")
                            ps = st8["ps"]
                            for j in (jj, jj + 1):
                                nc.tensor.matmul(
                                    ps[:],
                                    w_sb[:, x * DM + j * 128:
                                         x * DM + (j + 1) * 128],
                                    src[:, j * 512:(j + 1) * 512],
                                    start=(j == 0), stop=(j == NJ - 1))

                        def ev(dst=dstl, x=x, st8=st8):
                            nc.vector.tensor_copy(
                                dst[x][:, dc * 512:(dc + 1) * 512],
                                st8["ps"][:])
                        item = [lambda jj=jj, f=q2: f(jj)
                                for jj in range(0, NJ, 2)] + [ev]
                        fillers.append(item)
                        need[x].append(item)
                    st8 = {}

                    def v2q(u, x=x, st8=st8, xv=xv):
                        # one u-slice: 8 matmuls of 128 cols (~0.5us)
                        if u == 0:
                            st8["ps"] = psM.tile([128, 512], f32, tag="mm", name="ps")
                        ps = st8["ps"]
                        for j in range(NJ):
                            nc.tensor.matmul(
                                ps[:, u * 128:(u + 1) * 128],
                                xv[:, j * 512 + u * 128:
                                   j * 512 + (u + 1) * 128],
                                wv_sb[:, x * DM + j * 128:
                                      x * DM + (j + 1) * 128],
                                start=(j == 0), stop=(j == NJ - 1))

                    def vev(x=x, st8=st8):
                        nc.vector.tensor_copy(
                            v_sb[x].rearrange("p (t w) -> p t w", w=VW)
                            [:, 4 * dc:4 * dc + 4, 0:130]
                            .rearrange("p u (h w) -> p u h w", h=2)
                            [:, :, :, 0:64],
                            st8["ps"][:].rearrange(
                                "p (u h w) -> p u h w", u=4, h=2))
                    item = [lambda u=u, f=v2q: f(u)
                            for u in range(4)] + [vev]
                    fillers.append(item)
                    need[x].append(item)
                return need

            def _drain_items(items):
                while any(any(it is f for f in fillers) for it in items):
                    _pop_fillers(1)

            def _emit_norm_now(nx, nc_, not0, not1, nrcp):
                # rps broadcast matmul + fused multiply-evacuate into oT.
                # Emitted directly (never via the FIFO): the next pair's
                # first PV waits on the psO banks this releases, so its
                # instructions must precede that PV in program order.
                rps_ps = psM.tile([128, 512], f32, tag="mm")
                for h in (0, 1):
                    nc.tensor.matmul(rps_ps[h * 64:(h + 1) * 64, :],
                                     ones_sb[0:1, 0:64],
                                     nrcp[0:1, h * 512:(h + 1) * 512],
                                     start=True, stop=True)
                for h, otx in ((0, not0), (1, not1)):
                    rps_sb = smp.tile([64, 512], bf16, tag=f"rps{h}")
                    nc.vector.tensor_copy(rps_sb[:],
                                          rps_ps[h * 64:(h + 1) * 64, :])
                    nc.vector.tensor_tensor(
                        out=oT[nx][h * 64:h * 64 + 64,
                                   nc_ * 512:(nc_ + 1) * 512],
                        in0=otx[0:64, :], in1=rps_sb[:], op=mult)

            def _wo_fillers(wc, tail=False):
                # y rows for chunk wc: 4 t-tiles x 2 halves of 512
                for wt in range(4 * wc, 4 * wc + 4):
                    st8 = {}

                    def wo_half(wm, wt=wt, st8=st8, tail=tail):
                        if wm == 0:
                            st8["ysb"] = ysbp.tile([128, 1024], bf16, tag="ysb", name="ysb")
                        yps = psM.tile([128, 512], f32, tag="mm")
                        for x in range(2):
                            nc.tensor.matmul(
                                yps[:],
                                oT[x][:, wt * 128:(wt + 1) * 128],
                                wo_sb[:, x * DM + wm * 512:
                                      x * DM + (wm + 1) * 512],
                                start=(x == 0), stop=(x == 1))
                        if tail and wm == 1:  # spread drain over ACT too
                            nc.scalar.copy(
                                st8["ysb"][:, wm * 512:(wm + 1) * 512],
                                yps[:])
                        else:
                            nc.vector.tensor_copy(
                                st8["ysb"][:, wm * 512:(wm + 1) * 512],
                                yps[:])
                        if wm == 1:
                            nc.sync.dma_start(
                                out=y[wt * 128:(wt + 1) * 128, :],
                                in_=st8["ysb"][:])
                    fillers.append([lambda wm=wm, f=wo_half: f(wm)
                                    for wm in range(2)])

            def _pop_fillers(n):
                done = 0
                while fillers and done < n:
                    item = fillers[0]
                    item.pop(0)()
                    if not item:
                        fillers.pop(0)
                    done += 1

            # HAM warm-up: dependency-free dummy matmuls so the PE clock
            # is at 8/8 when the first projections arrive (covers the
            # initial input-DMA window; PE would be idle regardless).
            warm_ps = psO.tile([64, 512], f32, tag="ot0")
            for _ in range(20):
                nc.tensor.matmul(warm_ps[:], ones_sb[0:1, 0:64], ones_sb[:],
                                 start=True, stop=True)

            xt_cur = _emit_dmas(0)
            _late_const_dmas()
            # only pair a's chunk-0 projections run upfront; pair b's
            # drain inside pair a's rounds.  chunk 1's input DMA is
            # issued after chunk 0's so it doesn't steal queue bandwidth
            proj_need = _proj_fillers(xt_cur, 0)
            _drain_items(proj_need[0])
            xt_next = _emit_dmas(1)
            pending_norm = None

            for c in range(NC_Q):
                if c + 1 < NC_Q:
                    next_need = _proj_fillers(xt_next, c + 1)
                    if c + 2 < NC_Q:
                        xt_next = _emit_dmas(c + 2)
                else:
                    next_need = {0: [], 1: []}

                for x in range(2):  # pair a then pair b round streams
                    # this pair's projections must be fully emitted
                    # before its rounds reference qT/kT/v_sb (FIFO order
                    # keeps proj items ahead of any hazardous WO item)
                    _drain_items(proj_need[x])
                    n_t = 4 * c + 4

                    def _scores_exp(t):
                        st = psS.tile([128, 1024], f32, tag="st",
                                      name="st")
                        soff = max(0, (t - 4 * c) * 128) if trim else 0
                        for h in (0, 1):
                            hp = h * 64
                            nc.tensor.matmul(
                                st[:, h * 512 + soff:(h + 1) * 512],
                                kT[x][hp:hp + 64, t * 128:(t + 1) * 128],
                                qT[x][hp:hp + 64,
                                      c * 512 + soff:(c + 1) * 512],
                                start=True, stop=True)
                        pt = ptp.tile([128, 1024], bf16, tag="pt",
                                      name="pt")
                        nc.scalar.activation(pt[:], st[:], Exp,
                                             scale=float(SCALE))
                        if t >= 4 * c:  # diagonal tile: mask aligned block
                            dd = t - 4 * c
                            mw = 128 if trim else (dd + 1) * 128
                            for h in (0, 1):
                                nc.vector.tensor_tensor(
                                    out=pt[:, h * 512 + soff:
                                           h * 512 + soff + mw],
                                    in0=pt[:, h * 512 + soff:
                                           h * 512 + soff + mw],
                                    in1=masks_sb[:, dd * 512 + soff:
                                                 dd * 512 + soff + mw],
                                    op=mult)
                        return pt, soff

                    def _pv(t, pt, off):
                        for h, otx in ((0, ot0), (1, ot1)):
                            nc.tensor.matmul(
                                otx[0:65, off:512],
                                v_sb[x][:, t * VW + h * 65:
                                        t * VW + h * 65 + 65],
                                pt[:, h * 512 + off:(h + 1) * 512],
                                start=(t == n_t - 1), stop=(t == 0))

                    # first round's scores+exp go ahead of the previous
                    # pair's norm so ACT runs gaplessly through the
                    # switch; its PV waits on the psO banks norm frees.
                    t0 = n_t - 1
                    pt0, off0 = _scores_exp(t0)
                    if pending_norm is not None:
                        _emit_norm_now(*pending_norm)
                        pending_norm = None
                    ot0 = psO.tile([65, 512], f32, tag="ot0", name="ot0")
                    ot1 = psO.tile([65, 512], f32, tag="ot1", name="ot1")
                    # s_k tiles DESCENDING: widest PV first (start=True
                    # clears the bank; later narrower tiles accumulate
                    # where written) -> causally-trimmed ragged PV legal.
                    _pv(t0, pt0, off0)
                    _pop_fillers(3)
                    for t in range(n_t - 2, -1, -1):
                        pt, off = _scores_exp(t)
                        _pv(t, pt, off)
                        _pop_fillers(3)

                    # rowsum -> 1/r on ACT (stays in exp table set);
                    # everything else deferred to fillers.
                    lnr = smp.tile([1, 1024], f32, tag="lnr")
                    nc.scalar.activation(lnr[0:1, 0:512], ot0[64:65, :], Ln)
                    nc.scalar.activation(lnr[0:1, 512:1024], ot1[64:65, :],
                                         Ln)
                    rcp = smp.tile([1, 1024], bf16, tag="rcp")
                    nc.scalar.activation(rcp[:], lnr[:], Exp, scale=-1.0)
                    pending_norm = (x, c, ot0, ot1, rcp)
                # WO for this chunk drains during the next chunk's rounds
                _wo_fillers(c, tail=(c == NC_Q - 1))
                proj_need = next_need
            _emit_norm_now(*pending_norm)  # last pair's norm
            while fillers:  # tail drain
                _pop_fillers(8)
    if split_waits:
        _split_waits(nc, mybir)
    return nc


def _split_waits(nc, mybir):
    """This walrus build encodes at most ONE sync wait per instruction.
    Instructions with a single wait keep it inline (free); only multi-
    wait instructions get the extra waits hoisted onto same-engine NoOps
    issued immediately before — semantically identical: the sequencer
    blocks at the NoOp instead."""
    ctr = [0]
    for fn in nc.m.functions:
        for blk in fn.blocks:
            new_insts = []
            for ins in blk.instructions:
                si = getattr(ins, "sync_info", None)
                waits = list(si.on_wait) if si is not None and si.on_wait else []
                if len(waits) > 1:
                    for w in waits[:-1]:
                        ctr[0] += 1
                        nop = mybir.InstNoOp(name=f"WSPLIT-{ctr[0]}", ins=[], outs=[])
                        nop.engine = ins.engine
                        nop.sync_info = mybir.SyncInfo(on_wait=[w], on_update=[])
                        new_insts.append(nop)
                    ins.sync_info = mybir.SyncInfo(
                        on_wait=[waits[-1]], on_update=list(si.on_update or []))
                new_insts.append(ins)
            blk.instructions = new_insts


def _marshal(Q, K, V, WQ, WK, WV, WO):
    Q = np.asarray(Q, dtype=np.float32)
    K = np.asarray(K, dtype=np.float32)
    V = np.asarray(V, dtype=np.float32)
    WQ = np.asarray(WQ, dtype=np.float32)
    WK = np.asarray(WK, dtype=np.float32)
    WV = np.asarray(WV, dtype=np.float32)
    WO = np.asarray(WO, dtype=np.float32)

    import ml_dtypes
    bf = ml_dtypes.bfloat16
    # [B, DM, S] bf16, one batch slice per core
    xt = [np.ascontiguousarray(X.transpose(1, 2, 0)).astype(bf)
          for X in (Q, K, V)]

    masks = np.zeros((4, 128, 512), dtype=bf)
    kk = np.arange(128)[:, None]
    qq = np.arange(512)[None, :]
    for d in range(4):
        masks[d] = (d * 128 + kk <= qq).astype(bf)

    in_maps = []
    for core in range(NCORES):
        b = core // 4
        h0 = (core % 4) * HEADS_PER_CORE
        wql = np.stack([np.concatenate([WQ[h0 + 2 * x], WQ[h0 + 2 * x + 1]],
                                       axis=1) for x in range(2)]).astype(bf)
        wkl = np.stack([np.concatenate([WK[h0 + 2 * x], WK[h0 + 2 * x + 1]],
                                       axis=1) for x in range(2)]).astype(bf)
        wvl = np.stack([np.concatenate([WV[h0 + 2 * x], WV[h0 + 2 * x + 1]],
                                       axis=1) for x in range(2)]).astype(bf)
        wol = np.stack([WO[(h0 + 2 * x) * DV:(h0 + 2 * x + 2) * DV, :]
                        for x in range(2)]).astype(bf)
        in_maps.append({
            "xtq": xt[0][b], "xtk": xt[1][b], "xtv": xt[2][b],
            "wq": np.ascontiguousarray(wql),
            "wk": np.ascontiguousarray(wkl),
            "wv": np.ascontiguousarray(wvl),
            "wo": np.ascontiguousarray(wol),
            "masks": masks,
        })
    return in_maps


LAST_RESULTS = None


def kernel(Q, K, V, WQ, WK, WV, WO):
    global LAST_RESULTS
    from concourse.bass_utils import run_bass_kernel_spmd

    if "nc" not in _CACHE:
        _CACHE["nc"] = build_nc()
    nc = _CACHE["nc"]

    in_maps = _marshal(Q, K, V, WQ, WK, WV, WO)
    res = run_bass_kernel_spmd(nc, in_maps, core_ids=list(range(NCORES)))
    LAST_RESULTS = res
    out = np.zeros((S, B, DM), dtype=np.float32)
    for core, r in enumerate(res.results):
        out[:, core // 4, :] += np.asarray(r["y"]).astype(np.float32)
    return out
